# revision 41
# baseline (speedup 1.0000x reference)
"""Trainium2 Bass kernel for the 4-layer sum/product circuit (segment_reduce).

Strategy: shard batch (4096) across 8 cores (512 each), zero communication.
Node-major layout: every circuit array lives in HBM as [n_nodes, 512] fp16
rows (one row = one node's batch slice, 1KB). Each layer is a chunked SWDGE
dma_gather (1024 indices/call, round-robin over 4 SWDGE queues so all four
Q7 core pairs generate descriptors concurrently) with host-permuted indices
so the k legs of each output land in contiguous free-axis columns of one
partition. DVE does the k-leg sums as all-fp16 pairwise trees (4x perf
mode), ACT applies exp/ln, and HWDGE writes 4-chunk groups back in a
partition-major row order (node o at row (o%128)*(n/128)+o//128, host
remaps the next layer's indices to match) so each partition writes one
contiguous multi-KB block. GPSIMD only generates DMA descriptors.
"""

import math
import numpy as np
from contextlib import ExitStack

import concourse.bacc as bacc
import concourse.tile as tile
from concourse import bass, mybir
from concourse import library_config
from concourse.bass_utils import run_bass_kernel_spmd

N_CORES = 8
B = 4096
ELEM = B // N_CORES         # 512 batch per core = one gathered row

N_XENC = 2050
N_L1 = 8192
N_L2 = 4096
N_L3 = 8192
N_OUT = 2048

CHUNK_IDX = 1024            # gather indices per dma_gather (HW limit ~1024)
WG = 16                     # chunks per grouped write-back
NQ = 4                      # SWDGE queues (Q7 core pairs)

_EXP = mybir.ActivationFunctionType.Exp
_LN = mybir.ActivationFunctionType.Ln
_FP32 = mybir.dt.float32
_FP16 = mybir.dt.float16

# (idx_name, src_name, n_src, n_out, k, act, out_name, out_dtype, remap_src)
LAYERS = [
    ("g1", "xenc", N_XENC, N_L1, 4, _EXP, "e1", _FP16, False),
    ("g2", "e1", N_L1, N_L2, 8, _LN, "l2", _FP16, True),
    ("g3", "l2", N_L2, N_L3, 4, _EXP, "e3", _FP16, True),
    ("g4", "e3", N_L3, N_OUT, 8, _LN, "out", _FP16, True),
]


def _perm_wrap_idx(orig: np.ndarray, n_out: int, k: int) -> np.ndarray:
    """Permute [n_out, k] gather indices into dma_gather order and wrap.

    dma_gather writes gathered row i to (partition i%128, col i//128). We
    want output o's leg j at (p = o%128, col = (o//128)*k + j), i.e.
    i = ((o//128)*k + j)*128 + o%128, so the k legs of each output are
    contiguous columns within one partition.
    Returns the int16 [128, n_out*k//16] wrapped index tile.
    """
    og = orig.reshape(n_out // 128, 128, k)         # [o2, p, j]
    flat = og.transpose(0, 2, 1).reshape(-1)        # [(o2 k) p] -> i = c*128+p
    q = flat.shape[0]
    w = flat.reshape(q // 16, 16).T.astype(np.int16)  # [16, q/16]
    return np.tile(w, (8, 1))                       # [128, q/16]


def _log1mexp(x):
    # match reference (Maechler 2012) in f32
    x = x.astype(np.float32)
    with np.errstate(divide="ignore", invalid="ignore"):
        a = np.log(-np.expm1(x)).astype(np.float32)
        b = np.log1p(-np.exp(x)).astype(np.float32)
    return np.where(x > -math.log(2.0), a, b).astype(np.float32)


def _build(nc):
    i16 = mybir.dt.int16
    add = mybir.AluOpType.add

    xenc_d = nc.dram_tensor("xenc", [N_XENC, ELEM], _FP16,
                            kind="ExternalInput")
    e1_d = nc.dram_tensor("e1", [N_L1, ELEM], _FP16, kind="Internal")
    l2_d = nc.dram_tensor("l2", [N_L2, ELEM], _FP16, kind="Internal")
    e3_d = nc.dram_tensor("e3", [N_L3, ELEM], _FP16, kind="Internal")
    out_d = nc.dram_tensor("out", [N_OUT, ELEM], _FP16,
                           kind="ExternalOutput")
    tensors = {"xenc": xenc_d, "e1": e1_d, "l2": l2_d, "e3": e3_d,
               "out": out_d}

    idx_d = {}
    for lay in LAYERS:
        name, n_out, k = lay[0], lay[3], lay[4]
        idx_d[name] = nc.dram_tensor(f"{name}idx", [128, n_out * k // 16],
                                     i16, kind="ExternalInput").ap()

    with tile.TileContext(nc) as tc, ExitStack() as ctx:
        nc.gpsimd.load_library(library_config.mlp)
        idxp = ctx.enter_context(tc.tile_pool(name="idxp", bufs=1))
        gpool = ctx.enter_context(tc.tile_pool(name="gpool", bufs=10))
        apool = ctx.enter_context(tc.tile_pool(name="apool", bufs=4))
        rpool = ctx.enter_context(tc.tile_pool(name="rpool", bufs=2))

        idx_ts = {}
        for lay in LAYERS:
            name = lay[0]
            t = idxp.tile(list(idx_d[name].shape), i16, tag=f"idx_{name}")
            nc.sync.dma_start(t[:], idx_d[name][:])
            idx_ts[name] = t

        for idx_name, src_name, n_src, n_out, k, act, dst_name, out_dt, _ \
                in LAYERS:
            src_dt = tensors[src_name].dtype
            src_ap = tensors[src_name].ap()
            idx_t = idx_ts[idx_name]

            chunk_out = CHUNK_IDX // k              # output nodes per chunk
            oc = chunk_out // 128                   # output cols per chunk
            gcols = CHUNK_IDX // 128                # gather cols per chunk
            icols = CHUNK_IDX // 16                 # idx cols per chunk
            nchunks = n_out // chunk_out
            # write schedule: big overlapped groups early, then per-chunk
            # writes for the tail so the layer barrier clears right after
            # the last chunk's compute instead of behind a 4MB write
            wgs = [16] * (nchunks // 16)
            gstart = [0]
            for w in wgs:
                gstart.append(gstart[-1] + w)
            assert gstart[-1] == nchunks
            # p-major storage: node o at row (o%128)*(n_out/128) + o//128
            dst_view = tensors[dst_name].ap().rearrange(
                "(p n) e -> p n e", p=128)

            gi = 0
            for ci in range(nchunks):
                g = gpool.tile([128, gcols, ELEM], src_dt, tag="g")
                nc.gpsimd.dma_gather(
                    g[:], src_ap,
                    idx_t[:, ci * icols:(ci + 1) * icols],
                    CHUNK_IDX, CHUNK_IDX, ELEM,
                    queue_num=ci % NQ,
                )
                # all-fp16 pairwise tree (DVE 4x perf mode), one wide
                # strided tensor_tensor per tree level
                gv = g[:].rearrange("p (o k2 two) e -> p o k2 two e",
                                    k2=k // 2, two=2)
                acc = apool.tile([128, oc, k // 2, ELEM], _FP16, tag="acc")
                nc.vector.tensor_tensor(acc[:], gv[:, :, :, 0, :],
                                        gv[:, :, :, 1, :], add)
                m = k // 2
                while m > 1:
                    a2 = acc[:, :, :m, :].rearrange(
                        "p o (k2 two) e -> p o k2 two e", k2=m // 2, two=2)
                    nc.vector.tensor_tensor(acc[:, :, :m // 2, :],
                                            a2[:, :, :, 0, :],
                                            a2[:, :, :, 1, :], add)
                    m //= 2
                av = acc[:]
                if ci == gstart[gi]:
                    r = rpool.tile([128, wgs[gi] * oc, ELEM], out_dt,
                                   tag="r")
                wi = (ci - gstart[gi]) * oc
                nc.scalar.activation(r[:, wi:wi + oc, :], av[:, :, 0, :],
                                     act)
                if ci == gstart[gi + 1] - 1:
                    nc.sync.dma_start(
                        dst_view[:, gstart[gi] * oc:gstart[gi + 1] * oc, :],
                        r[:])
                    gi += 1
    nc.compile()
    return nc


_CACHED_NC = None
_LAST_IN_MAPS = None


def _remap(v: np.ndarray, n_src: int) -> np.ndarray:
    """Node id -> p-major storage row."""
    return (v % 128) * (n_src // 128) + v // 128


def kernel(pos, idx0, idx1, idx2, idx3):
    global _CACHED_NC, _LAST_IN_MAPS
    pos = np.asarray(pos, dtype=np.float32)

    # host-side input encoding: x_enc [2050, 4096]
    neg = _log1mexp(pos)
    n, b = pos.shape
    xenc = np.zeros((2 * n + 2, b), np.float32)
    xenc[1] = 0.0
    xenc[2::2] = pos
    xenc[3::2] = neg
    # row 0 is -inf in the reference but never gathered (idx0 >= 1); keep 0.

    idx_maps = {}
    for lay, arr in zip(LAYERS, (idx0, idx1, idx2, idx3)):
        name, n_src, n_out, k, remap = lay[0], lay[2], lay[3], lay[4], lay[8]
        v = np.asarray(arr).astype(np.int64)
        if remap:
            v = _remap(v, n_src)
        idx_maps[f"{name}idx"] = _perm_wrap_idx(v, n_out, k)

    if _CACHED_NC is None:
        _CACHED_NC = _build(bacc.Bacc("TRN2", target_bir_lowering=False,
                                      debug=False,
                                      num_swdge_queues=NQ,
                                      dynamic_dma_scratch_size=32768))
    nc = _CACHED_NC

    in_maps = []
    for c in range(N_CORES):
        sl = np.ascontiguousarray(
            xenc[:, c * ELEM:(c + 1) * ELEM].astype(np.float16))
        in_maps.append({"xenc": sl, **idx_maps})

    _LAST_IN_MAPS = in_maps
    res = run_bass_kernel_spmd(nc, in_maps, list(range(N_CORES)))
    out = np.empty((N_OUT, B), np.float32)
    for c in range(N_CORES):
        # p-major rows back to node order: row (o%128)*16 + o//128 -> o
        ot = res.results[c]["out"].astype(np.float32)
        ot = ot.reshape(128, N_OUT // 128, ELEM)
        out[:, c * ELEM:(c + 1) * ELEM] = \
            ot.transpose(1, 0, 2).reshape(N_OUT, ELEM)
    return out
